# revision 31
# baseline (speedup 1.0000x reference)
"""ContrastiveMagnitudeLoss on 8 Trainium2 NeuronCores (Bass/Tile), v3.

Strategy (sharding_hint: shard batch across cores, replicate target):
  - B=4096 rows of `predicted` are sharded 512/core; every core holds the
    full transposed `target`, so each core owns complete rows of the BxB
    distance matrix and the row-softmax needs no communication.
  - Cross products run on the PE in fp8e4 (DoubleRow perf mode packs two
    128-deep k-chunks per matmul at 0.5 cyc/row): PSUM accumulates
    X = p.t - tsq/2, with the tsq rank-2 term carried by a bf16 hi/lo
    ext matmul in the same accumulation group.  fp8 rounding of the
    cross term shifts the contrastive loss by ~8e-4 relative (measured
    against the f32 reference on the real data) - inside the 2e-3 gate.
  - ScalarE needs only TWO passes per element (vs ln/exp/exp):
      d = Sqrt(-2*X + psq_m)   (psq rides the per-partition bias)
      S += Exp(-10*d + b_m)    (b_m = 10*d_ii - 40, fused accum_out)
    All 8 Sqrt chains run first, then all 4 Exp chains, so the sqrt->exp
    activation-table switch happens exactly once (~2.7us).
  - The magnitude-loss numerator sum_d |p - t| runs on the otherwise-idle
    VectorE from bf16 copies of the shard: subtract + fused
    (|diff| + acc) via scalar_tensor_tensor(abs_max, add).
  - Inputs stream over all three DMA queues (qSync/qScalar HWDGE +
    qPool SWDGE, ~110 GB/s each): each queue carries one fp8 k-chunk
    pair of target in need-order column blocks, so the first matmul
    sweep unblocks after ~0.5 MB/queue.
  - Host does the O(B*D) input prep (transpose/cast/row stats) and the
    final O(B) reduction of per-row scalars; all O(B^2 D) and O(B^2)
    work runs on the NeuronCores.

Outputs per core: s_out [128,4] f32 softmax sums (col = m-tile),
l1_out [128,512] f32 per-contraction-partition |p-t| sums.
"""

import numpy as np
import ml_dtypes

BF16 = ml_dtypes.bfloat16
F8 = ml_dtypes.float8_e4m3fn

B = 4096
D = 768
NCORES = 8
BL = B // NCORES          # 512 rows per core
P = 128                   # partitions
NK = D // P               # 6 contraction chunks of 128
NCP = NK // 2             # 3 fp8 DoubleRow chunk pairs
NT = BL // P              # 4 m-tiles per core
HB = B // 2               # 2048: column half per PSUM tile
HDRW = NK * P + 32        # hdr: pt fp8 m-tile0 + 8 f32 bias bytes
TTW = [1024, 1024, 2048]  # tt need-order block widths
TTOFF = [0, 1024, 2048]
C_STAB = 40.0

_COMPILED = None
LAST_RESULTS = None


def _build_bass():
    return _build_bass_inner()


def _build_bass_inner():
    import concourse.mybir as mybir
    import concourse.tile as tile
    from concourse import bacc
    from contextlib import ExitStack

    f32 = mybir.dt.float32
    bf16 = mybir.dt.bfloat16
    fp8 = mybir.dt.float8e4

    nc = bacc.Bacc("TRN2", target_bir_lowering=False, debug=False,
                   num_devices=NCORES)

    hdr_d = nc.dram_tensor("hdr", [P, HDRW], fp8,
                           kind="ExternalInput").ap()
    ptr_d = nc.dram_tensor("ptr8", [P, (NT - 1) * NK * P], fp8,
                           kind="ExternalInput").ap()
    ttq_d = [nc.dram_tensor(f"ttq{j}", [P, 2 * B], fp8,
                            kind="ExternalInput").ap()
             for j in range(NCP)]
    tx_d = nc.dram_tensor("txb", [2, B], bf16, kind="ExternalInput").ap()
    ptb_d = nc.dram_tensor("ptb", [P, NK * BL], bf16,
                           kind="ExternalInput").ap()
    tsb_d = nc.dram_tensor("tsb", [P, NK * BL], bf16,
                           kind="ExternalInput").ap()
    s_d = nc.dram_tensor("s_out", [P, NT], f32, kind="ExternalOutput").ap()
    l1_d = nc.dram_tensor("l1_out", [P, BL], f32,
                          kind="ExternalOutput").ap()

    with tile.TileContext(nc) as tc, ExitStack() as ctx:
        const_pool = ctx.enter_context(tc.tile_pool(name="consts", bufs=1))
        work_pool = ctx.enter_context(tc.tile_pool(name="work", bufs=2))

        # tt SBUF layout is pair-block-major: pair j occupies cols
        # [j*2B, (j+1)*2B); inside, block b at 2*TTOFF[b] holds chunk
        # 2j's cols then chunk 2j+1's cols.  DMA src and dst are then
        # contiguous per partition (one fat descriptor per partition).
        tt8 = const_pool.tile([P, NK * B], fp8, name="tt8")

        def tt_rhs(cp, c0, w):
            # [P, 2, w] AP of chunk pair cp, columns [c0, c0+w)
            for off, bw in zip(TTOFF, TTW):
                if off <= c0 < off + bw:
                    break
            base = cp * 2 * B + 2 * off
            blk = tt8[:, base:base + 2 * bw].rearrange(
                "p (i c) -> p i c", i=2)
            return blk[:, :, c0 - off:c0 - off + w]
        hdr_sb = const_pool.tile([P, HDRW], fp8, name="hdr_sb")
        ptr_sb = const_pool.tile([P, (NT - 1) * NK * P], fp8, name="ptr_sb")
        tx_sb = const_pool.tile([2, B], bf16, name="tx_sb")
        ptb_sb = const_pool.tile([P, NK * BL], bf16, name="ptb_sb")
        tsb_sb = const_pool.tile([P, NK * BL], bf16, name="tsb_sb")
        ones_sb = const_pool.tile([2, P], bf16, name="ones_sb")
        warm_sb = const_pool.tile([P, 512], bf16, name="warm_sb")
        s_sb = const_pool.tile([P, NT], f32, name="s_sb")
        d_sb = [const_pool.tile([P, B], f32, name=f"d{t}")
                for t in range(NT)]
        escr = const_pool.tile([P, B], f32, name="escr")

        psq_sb = hdr_sb[:, NK * P:NK * P + 16].bitcast(f32)
        bexp_sb = hdr_sb[:, NK * P + 16:NK * P + 32].bitcast(f32)
        bgate = const_pool.tile([P, NT], f32, name="bgate")
        zgate = const_pool.tile([P, NT], f32, name="zgate")

        # memsets on the (early-idle) VectorE so the PE warm-up is not
        # gated by gpsimd's DMA dispatch stream
        nc.vector.memset(warm_sb, 0.0)
        nc.vector.memset(ones_sb, 1.0)

        # ---- input DMAs across the 3 queues, in need-order ----
        def dma_tt(eng, j, b):
            off, w = TTOFF[b], TTW[b]
            base = j * 2 * B + 2 * off
            eng.dma_start(tt8[:, base:base + 2 * w],
                          ttq_d[j][:, 2 * off:2 * (off + w)])

        # The 16 SDMA engines fair-share across the 3 rings' HEAD
        # transfers, so every queue must enqueue in global need order:
        # h0 blocks first, h1 next, bulky L1 inputs last.  ptr8 rides
        # the sync ring in per-m-tile slices timed to sweeps t1..t3.
        nc.sync.dma_start(hdr_sb, hdr_d)
        nc.gpsimd.dma_start(tx_sb, tx_d)
        dma_tt(nc.sync, 0, 0)
        dma_tt(nc.scalar, 1, 0)
        dma_tt(nc.gpsimd, 2, 0)
        nc.sync.dma_start(ptr_sb[:, :NK * P], ptr_d[:, :NK * P])
        dma_tt(nc.sync, 0, 1)
        dma_tt(nc.scalar, 1, 1)
        dma_tt(nc.gpsimd, 2, 1)
        nc.sync.dma_start(ptr_sb[:, NK * P:2 * NK * P],
                          ptr_d[:, NK * P:2 * NK * P])
        nc.sync.dma_start(ptr_sb[:, 2 * NK * P:], ptr_d[:, 2 * NK * P:])
        dma_tt(nc.sync, 0, 2)           # h1 blocks
        dma_tt(nc.scalar, 1, 2)
        dma_tt(nc.gpsimd, 2, 2)
        nc.scalar.dma_start(ptb_sb, ptb_d)
        nc.scalar.dma_start(tsb_sb, tsb_d)

        def pt_pair(t, cp):
            if t == 0:
                ap = hdr_sb[:, 2 * cp * P:(2 * cp + 2) * P]
            else:
                base = (t - 1) * NK * P + 2 * cp * P
                ap = ptr_sb[:, base:base + 2 * P]
            return ap.rearrange("p (i m) -> p i m", i=2)

        # ---- magnitude loss on VectorE: acc += |p - t| per chunk ----
        acc = None
        for k in range(NK):
            diff = work_pool.tile([P, BL], bf16, name="diff", tag="diff")
            nc.vector.tensor_tensor(diff, ptb_sb[:, k * BL:(k + 1) * BL],
                                    tsb_sb[:, k * BL:(k + 1) * BL],
                                    op=mybir.AluOpType.subtract)
            ndiff = work_pool.tile([P, BL], bf16, name="ndiff", tag="ndiff")
            nc.vector.tensor_scalar(ndiff, diff, -1.0, None,
                                    op0=mybir.AluOpType.mult)
            absd = work_pool.tile([P, BL], f32, name="absd", tag="absd",
                                  bufs=3)
            nc.vector.tensor_tensor(absd, diff, ndiff,
                                    op=mybir.AluOpType.max)
            if acc is None:
                acc = absd
            else:
                nacc = work_pool.tile([P, BL], f32, name="nacc", tag="absd",
                                      bufs=3)
                nc.vector.tensor_tensor(nacc, acc, absd,
                                        op=mybir.AluOpType.add)
                acc = nacc
        nc.gpsimd.dma_start(l1_d, acc)

        # ---- main pipeline ----
        with tc.tile_pool(name="psum_x", bufs=2, space="PSUM") as psum_x:
            # PE HAM warm-up: keep the PE busy through the ~9us DMA
            # pre-roll so the 1.2 -> 2.4 GHz clock gate is open (and does
            # not re-close) when the first real matmul issues.
            warm_ps = psum_x.tile([P, P], f32, name="warm_ps", tag="xq")
            for _ in range(55):
                nc.tensor.matmul(warm_ps, lhsT=warm_sb[:, :P],
                                 rhs=warm_sb[:, :P], start=True, stop=True)
            CPORD = [1, 2, 0]   # chunk pairs ordered by queue arrival
            for h in range(2):
                for t in range(NT):
                    xq = psum_x.tile([P, HB], f32, name="xq", tag="xq")
                    # Zero-weight matmuls woven between the stages of
                    # the input-stream-gated early sweeps keep the PE
                    # duty cycle high, so the HAM busy-window stays open
                    # and the clock gate holds 2.4 GHz through the
                    # trickle phase (losing it costs ~14us of 1.2 GHz
                    # sweeps while it re-opens).
                    nfill = {(0, 0): 6, (0, 1): 2}.get((h, t), 0)
                    if h == 1 and t >= 2:
                        # Dummy weight loads busy the PE while it waits
                        # for its PSUM slot (the sqrt chain behind the
                        # woven E0 below) without touching PSUM, so the
                        # HAM clock gate stays open for the final sweeps.
                        for _ in range(14):
                            nc.tensor.ldweights(warm_sb[:, :P])
                    for ci, cp in enumerate(CPORD):
                        for f in range(nfill):
                            nc.tensor.matmul(
                                xq[:, 0:512], lhsT=warm_sb[:, :P],
                                rhs=warm_sb,
                                start=(ci == 0 and f == 0), stop=False)
                        for j in range(4):
                            c0 = h * HB + j * 512
                            nc.tensor.matmul(
                                xq[:, j * 512:(j + 1) * 512],
                                lhsT=pt_pair(t, cp),
                                rhs=tt_rhs(cp, c0, 512),
                                start=(ci == 0 and
                                       (j > 0 or nfill == 0)),
                                stop=False,
                                perf_mode=mybir.MatmulPerfMode.DoubleRow)
                    for j in range(4):
                        c0 = h * HB + j * 512
                        nc.tensor.matmul(
                            xq[:, j * 512:(j + 1) * 512],
                            lhsT=ones_sb, rhs=tx_sb[:, c0:c0 + 512],
                            start=False, stop=True)
                    nc.scalar.activation(
                        d_sb[t][:, h * HB:(h + 1) * HB], xq,
                        mybir.ActivationFunctionType.Sqrt,
                        scale=-2.0, bias=psq_sb[:, t:t + 1])
            # All Exp chains are gated (via a dummy dependency through
            # d_sb[3] into their bias tile) to run as one block after
            # the LAST sqrt chain.  Weaving any of them earlier inserts
            # a ~6.3us exp+table-switch bubble into the S-chain ladder,
            # which stalls the PE on PSUM recycling (2 slots) long
            # enough to close the HAM clock gate.
            nc.vector.tensor_scalar(zgate, d_sb[NT - 1][:, B - NT:B], 0.0,
                                    None, op0=mybir.AluOpType.mult)
            nc.vector.tensor_tensor(bgate, bexp_sb, zgate,
                                    op=mybir.AluOpType.add)
            for t in range(NT):
                bias = bexp_sb if t == 0 else bgate
                nc.scalar.activation(
                    escr, d_sb[t],
                    mybir.ActivationFunctionType.Exp,
                    scale=-10.0, bias=bias[:, t:t + 1],
                    accum_out=s_sb[:, t:t + 1])
            nc.scalar.dma_start(s_d, s_sb)

    nc.compile()
    return nc


def _get_compiled():
    global _COMPILED
    if _COMPILED is None:
        _COMPILED = _build_bass()
    return _COMPILED


def _split_bf16(v):
    hi = v.astype(np.float32).astype(BF16)
    lo = (v.astype(np.float32) - hi.astype(np.float32)).astype(BF16)
    return hi, lo


def kernel(predicted, target):
    global LAST_RESULTS
    from concourse.bass_utils import run_bass_kernel_spmd

    p = np.ascontiguousarray(np.asarray(predicted, dtype=np.float32))
    t = np.ascontiguousarray(np.asarray(target, dtype=np.float32))
    assert p.shape == (B, D) and t.shape == (B, D)

    # host-side O(B*D) row stats (input prep for the device program)
    p64 = p.astype(np.float64)
    t64 = t.astype(np.float64)
    psq = (p64 * p64).sum(1)
    tsq = (t64 * t64).sum(1)
    tmag = np.abs(t64).sum(1)
    dii = np.sqrt(((p64 - t64) ** 2).sum(1))

    # target, transposed, fp8 chunks striped across the 3 queues in
    # need-order column blocks (chunk pair j -> queue j)
    ttT = np.ascontiguousarray(t.T)                       # [768, 4096] f32
    tt8 = ttT.astype(F8).reshape(NK, P, B)
    ttq = []
    for j in range(NCP):
        pair = tt8[2 * j:2 * j + 2]                       # [2, 128, B]
        parts = [np.ascontiguousarray(pair[:, :, off:off + w]
                                      .transpose(1, 0, 2)).reshape(P, 2 * w)
                 for off, w in zip(TTOFF, TTW)]
        ttq.append(np.ascontiguousarray(np.concatenate(parts, axis=1)))
    hi, lo = _split_bf16(-0.5 * tsq)
    txb = np.ascontiguousarray(np.stack([hi, lo]))        # [2, B] bf16
    tsb_all = ttT.astype(BF16).reshape(NK, P, B)

    in_maps = []
    for c in range(NCORES):
        sl = slice(c * BL, (c + 1) * BL)
        pT8 = np.ascontiguousarray(p[sl].T).astype(F8).reshape(NK, P, BL)
        hdr = np.zeros((P, HDRW), dtype=F8)
        hdr[:, :NK * P] = (pT8[:, :, :P].transpose(1, 0, 2)
                           .reshape(P, NK * P))
        psq4 = np.ascontiguousarray(
            psq[sl].astype(np.float32).reshape(NT, P).T)   # [128, 4]
        bexp = np.ascontiguousarray(
            (10.0 * dii[sl] - C_STAB).astype(np.float32).reshape(NT, P).T)
        hdr.view(np.uint8)[:, NK * P:NK * P + 16] = psq4.view(np.uint8)
        hdr.view(np.uint8)[:, NK * P + 16:NK * P + 32] = bexp.view(np.uint8)
        ptr8 = np.zeros((P, (NT - 1) * NK * P), dtype=F8)
        for ti in range(1, NT):
            ptr8[:, (ti - 1) * NK * P:ti * NK * P] = (
                pT8[:, :, ti * P:(ti + 1) * P].transpose(1, 0, 2)
                .reshape(P, NK * P))
        ptb = np.ascontiguousarray(
            p[sl].T.astype(BF16).reshape(NK, P, BL)
            .transpose(1, 0, 2)).reshape(P, NK * BL)
        tsb = np.ascontiguousarray(
            t[sl].T.astype(BF16).reshape(NK, P, BL)
            .transpose(1, 0, 2)).reshape(P, NK * BL)
        m = {"hdr": hdr, "ptr8": ptr8, "txb": txb,
             "ptb": ptb, "tsb": tsb}
        for j in range(NCP):
            m[f"ttq{j}"] = ttq[j]
        in_maps.append(m)

    nc = _get_compiled()
    res = run_bass_kernel_spmd(nc, in_maps, core_ids=list(range(NCORES)))
    LAST_RESULTS = res

    S = np.empty(B, dtype=np.float64)
    l1 = np.empty(B, dtype=np.float64)
    for c in range(NCORES):
        out = res.results[c]
        S[c * BL:(c + 1) * BL] = (
            out["s_out"].astype(np.float64).T.reshape(BL))
        l1[c * BL:(c + 1) * BL] = out["l1_out"].astype(np.float64).sum(0)

    contrastive = float(np.log(S).mean() + C_STAB)
    magnitude = float((l1 / tmag).mean())
    total = 0.5 * contrastive + 0.5 * magnitude
    return (np.float32(total), np.float32(contrastive), np.float32(magnitude))
